# revision 1
# baseline (speedup 1.0000x reference)
"""AtomAttention Trainium2 kernel.

reference:
    bias = adj + dist + coulomb                      # [B, N, N]
    q = m @ Wq.T + bq; k = m @ Wk.T + bk; v = m @ Wv.T + bv
    attn = softmax(q @ k.T / sqrt(H) + bias, axis=-1)
    out  = attn @ v + m                              # [B, N, H]

B=16, N=1024, H=128.  Data-parallel over batch: 2 batches per core on 8
NeuronCores.  Per core ~26 MB of HBM reads (dominated by the three bias
tensors) -> memory-bound.

Layout strategy (all chosen so no on-device transposes are needed):
  - host passes m.T per batch, plus host-transposed bias tensors.
  - qT[h,n] = (scale*Wq.T).T-style matmul with mT as moving operand.
  - scores are computed in S.T layout [m, n] (m on partitions), so the
    bias tiles DMA straight in from the host-transposed adj/dist/coulomb.
  - softmax over m = partition dim: exp (no max subtraction; scores are
    bounded) then the denominator comes out of the PV matmul for free via
    a ones-column appended to v.  Normalization is a per-partition
    reciprocal scale on the PV output (partition dim = n there).
  - bv is folded in after normalization (sum_m P = 1), together with the
    residual m.
"""

import sys
import types

import numpy as np

B, N, H = 16, 1024, 128
NB = N // 128  # 8 row blocks
BPC = 2        # batches per core
NCORES = 8

_CACHE = {}


def _install_ntff_hook():
    """The agent image's antenv lacks axon_hooks; register the NTFF
    profiling hook manually so trace=True yields exec_time_ns."""
    if "antenv.axon_hooks" in sys.modules:
        return
    try:
        import trn_agent_boot.trn_boot as tb

        hook = tb._ntff_profile_via_ctypes("/opt/axon/libaxon_pjrt.so")
    except Exception:
        hook = None
    mod = types.ModuleType("antenv.axon_hooks")
    mod.get_axon_ntff_profile_hook = lambda: hook
    mod.set_axon_ntff_profile_hook = lambda h: None
    sys.modules["antenv.axon_hooks"] = mod


def _build():
    if "nc" in _CACHE:
        return _CACHE["nc"]
    import concourse.bass as bass
    from concourse import bacc, mybir, tile

    f32 = mybir.dt.float32
    bf16 = mybir.dt.bfloat16
    ts = bass.ts

    nc = bacc.Bacc("TRN2", target_bir_lowering=False, debug=False)

    NCH = 4  # bias chunks per batch (2 row-blocks each)
    # m is shipped twice (transposed for QKV, natural for the residual) but
    # in bf16, so total m bytes == one f32 copy.  QKV compute is bf16 on
    # device anyway; the residual in bf16 costs ~1e-3 rel err (gate 2e-2).
    mT = nc.dram_tensor("mT", [BPC, 128, N], bf16, kind="ExternalInput")
    mn_d = nc.dram_tensor("mn", [BPC, N, H], bf16, kind="ExternalInput")
    # host-interleaved adjT/distT/coulT, s-major: each half-chunk's three
    # tensors are contiguous so casts are per-half contiguous (2x mode)
    bias_d = nc.dram_tensor("biasT", [BPC, NCH, 2, 3, 128, N], f32,
                            kind="ExternalInput")
    wq_d = nc.dram_tensor("wq", [H, H], f32, kind="ExternalInput")
    wk_d = nc.dram_tensor("wk", [H, H], f32, kind="ExternalInput")
    wv_d = nc.dram_tensor("wv", [H, H], f32, kind="ExternalInput")
    bq_d = nc.dram_tensor("bq", [H, 1], f32, kind="ExternalInput")
    bk_d = nc.dram_tensor("bk", [H, 1], f32, kind="ExternalInput")
    bv_d = nc.dram_tensor("bv", [1, H], bf16, kind="ExternalInput")
    id_d = nc.dram_tensor("ident", [128, 128], bf16, kind="ExternalInput")
    out_d = nc.dram_tensor("out", [BPC, N, H], f32, kind="ExternalOutput")

    # [b, (i p), h] -> [b, p, i, h] so a [128, 8, 128] SBUF tile holds a
    # whole batch of m in natural orientation (p = row-within-block).
    mn_r = mn_d.rearrange("b (i p) h -> b p i h", p=128)
    out_r = out_d.rearrange("b (i p) h -> b p i h", p=128)
    bias_r = bias_d.rearrange("b c s t p n -> b c p (s t) n")

    Exp = mybir.ActivationFunctionType.Exp

    with tile.TileContext(nc) as tc:
        with (
            tc.tile_pool(name="const", bufs=1) as const,
            tc.tile_pool(name="big", bufs=4) as big,
            tc.tile_pool(name="bigb", bufs=3) as bigb,
            tc.tile_pool(name="sb", bufs=2) as sb,
            tc.tile_pool(name="work", bufs=6) as work,
            tc.tile_pool(name="epool", bufs=3) as epool,
            tc.tile_pool(name="pqk", bufs=2, space="PSUM") as pqk,
            tc.tile_pool(name="po", bufs=4, space="PSUM") as pop,
        ):
            # ---- one-time constants (tiny) first on the scalar ring,
            # then the m loads; big bias streams own the sync ring ----
            wq_f = const.tile([128, 128], f32)
            wk_f = const.tile([128, 128], f32)
            wv_f = const.tile([128, 128], f32)
            nc.scalar.dma_start(out=wq_f, in_=wq_d[:, :])
            nc.scalar.dma_start(out=wk_f, in_=wk_d[:, :])
            nc.scalar.dma_start(out=wv_f, in_=wv_d[:, :])
            wq_b = const.tile([128, 128], bf16)
            wk_b = const.tile([128, 128], bf16)
            wv_b = const.tile([128, 128], bf16)
            nc.vector.tensor_copy(wq_b, wq_f)
            nc.vector.tensor_copy(wk_b, wk_f)
            nc.vector.tensor_copy(wv_b, wv_f)
            bq_s = const.tile([128, 1], f32)
            bk_s = const.tile([128, 1], f32)
            nc.scalar.dma_start(out=bq_s, in_=bq_d[:, :])
            nc.scalar.dma_start(out=bk_s, in_=bk_d[:, :])
            # bv broadcast across partitions: [1,128] dram -> [128,128]
            bvb = const.tile([128, 128], bf16)
            bv_ap = bv_d[:, :]
            bv_bcast = bass.AP(
                tensor=bv_ap.tensor,
                offset=bv_ap.offset,
                ap=[[0, 128]] + list(bv_ap.ap[1:]),
            )
            nc.gpsimd.dma_start(out=bvb, in_=bv_bcast)
            # identity (bf16) for PE-side bias adds: I.T @ X accumulates X
            ident = const.tile([128, 128], bf16)
            nc.scalar.dma_start(out=ident, in_=id_d[:, :])
            # explicit zero bias for Exp: avoids the const-AP table machinery
            # (and its preamble TENSOR_LOADs on the critical startup chain)
            zb = const.tile([128, 1], f32)
            nc.vector.memset(zb, 0.0)

            # ---- m loads (bf16) + residual prep for both batches ----
            mT_bs, mb_ts = [], []
            for b in range(BPC):
                mT_b = sb.tile([128, N], bf16, name=f"mT_b{b}", tag="mT_b")
                nc.scalar.dma_start(out=mT_b, in_=mT[b])
                mn_t = sb.tile([128, NB, H], bf16, name=f"mn{b}", tag="mn")
                nc.scalar.dma_start(out=mn_t, in_=mn_r[b])
                # residual + bv, pre-summed (bf16 in, f32 out)
                mb_t = sb.tile([128, NB, H], f32, name=f"mb{b}", tag="mb")
                for i in range(NB):
                    nc.gpsimd.tensor_add(mb_t[:, i], mn_t[:, i], bvb)
                mT_bs.append(mT_b)
                mb_ts.append(mb_t)

            for b in range(BPC):
                mT_b = mT_bs[b]
                mb_t = mb_ts[b]

                # ---- qT / kT : [h, n] with h on partitions ----
                ps_q = pqk.tile([128, N], f32, name=f"ps_q{b}", tag="pqk")
                nc.tensor.matmul(ps_q[:, 0:512], lhsT=wq_b, rhs=mT_b[:, 0:512],
                                 start=True, stop=True)
                nc.tensor.matmul(ps_q[:, 512:1024], lhsT=wq_b,
                                 rhs=mT_b[:, 512:1024], start=True, stop=True)
                qT = sb.tile([128, N], bf16, name=f"qT{b}", tag="qT")
                nc.vector.tensor_scalar_add(qT[:, 0:512], ps_q[:, 0:512], bq_s)
                nc.vector.tensor_scalar_add(qT[:, 512:1024], ps_q[:, 512:1024],
                                            bq_s)

                ps_k = pqk.tile([128, N], f32, name=f"ps_k{b}", tag="pqk")
                nc.tensor.matmul(ps_k[:, 0:512], lhsT=wk_b, rhs=mT_b[:, 0:512],
                                 start=True, stop=True)
                nc.tensor.matmul(ps_k[:, 512:1024], lhsT=wk_b,
                                 rhs=mT_b[:, 512:1024], start=True, stop=True)
                kT = sb.tile([128, N], bf16, name=f"kT{b}", tag="kT")
                nc.vector.tensor_scalar_add(kT[:, 0:512], ps_k[:, 0:512], bk_s)
                nc.vector.tensor_scalar_add(kT[:, 512:1024], ps_k[:, 512:1024],
                                            bk_s)

                # ---- v (natural [m, h] layout) + ones column ----
                v_aug = sb.tile([128, NB, 132], bf16, name=f"v{b}", tag="v")
                nc.vector.memset(v_aug[:, :, 128:129], 1.0)
                for ci in range(NB):
                    ps_v = pqk.tile([128, 128], f32, name=f"ps_v{b}_{ci}",
                                    tag="pqk")
                    nc.tensor.matmul(ps_v, lhsT=mT_b[:, ts(ci, 128)], rhs=wv_b,
                                     start=True, stop=True)
                    nc.scalar.copy(v_aug[:, ci, 0:128], ps_v)

                # ---- PV accumulators: all 8 n-blocks live in PSUM across
                # the whole batch (2 blocks per bank), so PV matmuls run
                # per-chunk right after each exp instead of as a tail ----
                ps_os = [
                    pop.tile([128, 2, 132], f32, name=f"ps_o{b}_{p}", tag="po")
                    for p in range(NB // 2)
                ]

                # ---- scores (S.T layout) + bias + exp + PV per chunk ----
                # bias tensors load f32 on fast HWDGE DMA, get cast to bf16
                # on DVE (single-src 2x mode), and are added to the qk
                # scores on the TensorEngine via identity-matmul PSUM
                # accumulation; exp reads PSUM.
                for c in range(NCH):
                    bt_f = big.tile([128, 2, 3, N], f32, name=f"bt{b}_{c}",
                                    tag="a")
                    bt_b = bigb.tile([128, 2, 3, N], bf16, name=f"bb{b}_{c}",
                                     tag="ab")
                    nc.sync.dma_start(out=bt_f, in_=bias_r[b, c])
                    E = epool.tile([128, 2, N], bf16, name=f"E{b}_{c}", tag="E")
                    for s in range(2):
                        # per-half contiguous cast: halves the cast latency
                        # on the critical (last-chunk) chain
                        nc.vector.tensor_copy(bt_b[:, s], bt_f[:, s])
                        j = 2 * c + s
                        ps_s = pqk.tile([128, N], f32, name=f"ps_s{b}_{j}",
                                        tag="pqk")
                        for h in range(2):
                            hs = slice(512 * h, 512 * (h + 1))
                            nc.tensor.matmul(ps_s[:, hs],
                                             lhsT=kT[:, ts(j, 128)],
                                             rhs=qT[:, hs], start=True,
                                             stop=False)
                        for t in range(3):
                            for h in range(2):
                                hs = slice(512 * h, 512 * (h + 1))
                                nc.tensor.matmul(ps_s[:, hs], lhsT=ident,
                                                 rhs=bt_b[:, s, t, hs],
                                                 start=False, stop=(t == 2))
                        nc.scalar.activation(out=E[:, s], in_=ps_s, func=Exp,
                                             bias=zb)
                    for s in range(2):
                        j = 2 * c + s
                        for i in range(NB):
                            # start=True clears the whole PSUM bank, so only
                            # the bank's first matmul (even half, j==0) sets
                            # it; the odd half's first write lands on cleared
                            # has_written bits and overwrites.
                            nc.tensor.matmul(
                                ps_os[i // 2][:, i % 2, 0:129],
                                lhsT=E[:, s, ts(i, 128)],
                                rhs=v_aug[:, j, 0:129],
                                start=(j == 0 and i % 2 == 0),
                                stop=(j == NB - 1), skip_group_check=True)

                # ---- normalization + residual; muls split ACT/DVE and the
                # out-store issued per block-pair so the tail isn't
                # serialized on one engine or one big store ----
                ob = sb.tile([128, NB, H], f32, name=f"ob{b}", tag="ob")
                for i in range(NB):
                    ps_o = ps_os[i // 2][:, i % 2]
                    r = work.tile([128, 1], f32, name=f"r{b}_{i}", tag="r")
                    nc.vector.reciprocal(r, ps_o[:, 128:129])
                    o1 = work.tile([128, 128], f32, name=f"o1_{b}_{i}",
                                   tag="o1")
                    if i % 2 == 0:
                        nc.scalar.mul(o1, ps_o[:, 0:128], r)
                    else:
                        nc.vector.tensor_scalar_mul(o1, ps_o[:, 0:128], r)
                    nc.gpsimd.tensor_add(ob[:, i], o1, mb_t[:, i])
                    if i % 2 == 1:
                        nc.scalar.dma_start(out=out_r[b, :, i - 1:i + 1],
                                            in_=ob[:, i - 1:i + 1])

    nc.compile()
    _CACHE["nc"] = nc
    return nc


def _shard_inputs(m, adj, dist, coulomb, Wq, bq, Wk, bk, Wv, bv):
    scale = 1.0 / np.sqrt(np.float32(H))
    wqT = np.ascontiguousarray(Wq.T * scale).astype(np.float32)
    wkT = np.ascontiguousarray(Wk.T).astype(np.float32)
    wvT = np.ascontiguousarray(Wv.T).astype(np.float32)
    import ml_dtypes

    bq_s = (bq * scale).astype(np.float32).reshape(H, 1)
    bk_s = bk.astype(np.float32).reshape(H, 1)
    bv_s = bv.astype(ml_dtypes.bfloat16).reshape(1, H)

    # both m copies in bf16: combined == one f32 copy of m
    mT = np.ascontiguousarray(np.swapaxes(m, 1, 2)).astype(ml_dtypes.bfloat16)
    mn_b = np.ascontiguousarray(m).astype(ml_dtypes.bfloat16)
    adjT = np.swapaxes(adj, 1, 2)
    distT = np.swapaxes(dist, 1, 2)
    coulT = np.swapaxes(coulomb, 1, 2)
    # interleave the three (transposed) bias tensors per half-chunk:
    # [B, NCH, 2, 3, 128, N] contiguous
    NCH = 4
    stacked = np.stack(
        [t.reshape(B, NCH, 2, 128, N) for t in (adjT, distT, coulT)], axis=3
    )
    biasT = np.ascontiguousarray(stacked)

    ident = np.eye(128).astype(ml_dtypes.bfloat16)

    in_maps = []
    for c in range(NCORES):
        sl = slice(c * BPC, (c + 1) * BPC)
        in_maps.append({
            "mT": mT[sl],
            "mn": mn_b[sl],
            "biasT": biasT[sl],
            "wq": wqT, "wk": wkT, "wv": wvT,
            "bq": bq_s, "bk": bk_s, "bv": bv_s,
            "ident": ident,
        })
    return in_maps


def run(trace=False, **inputs):
    _install_ntff_hook()
    from concourse.bass_utils import run_bass_kernel_spmd

    nc = _build()
    in_maps = _shard_inputs(**inputs)
    try:
        res = run_bass_kernel_spmd(nc, in_maps, core_ids=list(range(NCORES)),
                                   trace=trace)
    except Exception:
        # transient device errors (NRT_EXEC_UNIT_UNRECOVERABLE) have been
        # observed on this fabric; one retry usually succeeds
        res = run_bass_kernel_spmd(nc, in_maps, core_ids=list(range(NCORES)),
                                   trace=trace)
    out = np.concatenate([r["out"] for r in res.results], axis=0)
    return out, res


def kernel(**inputs):
    inputs = {k: np.asarray(v) for k, v in inputs.items()}
    out, _ = run(trace=False, **inputs)
    return out



# revision 2
# speedup vs baseline: 1.0776x; 1.0776x over previous
"""AtomAttention Trainium2 kernel (fp8 DoubleRow version).

reference:
    bias = adj + dist + coulomb                      # [B, N, N]
    q = m @ Wq.T + bq; k = m @ Wk.T + bk; v = m @ Wv.T + bv
    attn = softmax(q @ k.T / sqrt(H) + bias, axis=-1)
    out  = attn @ v + m                              # [B, N, H]

B=16, N=1024, H=128.  Data-parallel over batch: 2 batches per core on 8
NeuronCores.  The bias tensors dominate HBM traffic, so they are shipped
pre-scaled by sqrt(H) in fp8e4m3 (6 MB/core instead of 24 MB f32); the
1/sqrt(H) score scale is re-applied inside the exp activation
(exp(scale*psum - C)), which also lets q/k stay at their natural range
for fp8.  A global shift C keeps exp outputs inside fp8 range; softmax
normalization cancels it exactly.

Compute layout (scores in S.T layout [m, n], softmax over partitions):
  - fp8 DoubleRow matmuls pair two contractions per pass (0.5 cyc/row):
      DR1: (kT_j | I) x (qT | b0_j)  -> k.T q + b0   (QK fused w/ bias add)
      DR2: (I | I)    x (b1_j | b2_j) -> + b1 + b2
    so the entire scores+bias block costs 1.0 row-pass, vs 4.0 for the
    bf16 matmul + 3 identity-add scheme.
  - exp on ACT reads PSUM, writes fp8 E; denominator comes from a ones
    column appended to v (also fp8), accumulated in the PV DoubleRows
    which pair consecutive j-blocks.
  - qT is written (PSUM->fp8) directly into slot 0 of each streaming
    bias chunk buffer so the DR1 moving operand (qT, b0) is one strided
    two-plane access pattern.
  - normalization is a per-partition reciprocal scale; residual m + bv
    pre-summed in bf16; output stored bf16 and upcast on host.

Measured rel err ~5e-3 (gate 2e-2).
"""

import sys
import types

import numpy as np

B, N, H = 16, 1024, 128
NB = N // 128   # 8 row blocks
NCH = 4         # chunks per batch, 2 blocks each
BPC = 2         # batches per core
NCORES = 8
SCALE = float(1.0 / np.sqrt(np.float32(H)))   # 0.08838835
INV = float(np.sqrt(np.float32(H)))           # 11.313708
CSHIFT = 3.25                                 # global exp shift

_CACHE = {}


def _install_ntff_hook():
    """The agent image's antenv lacks axon_hooks; register the NTFF
    profiling hook manually so trace=True yields exec_time_ns."""
    if "antenv.axon_hooks" in sys.modules:
        return
    try:
        import trn_agent_boot.trn_boot as tb

        hook = tb._ntff_profile_via_ctypes("/opt/axon/libaxon_pjrt.so")
    except Exception:
        hook = None
    mod = types.ModuleType("antenv.axon_hooks")
    mod.get_axon_ntff_profile_hook = lambda: hook
    mod.set_axon_ntff_profile_hook = lambda h: None
    sys.modules["antenv.axon_hooks"] = mod


def _build():
    if "nc" in _CACHE:
        return _CACHE["nc"]
    import concourse.bass as bass
    from concourse import bacc, mybir, tile

    f32 = mybir.dt.float32
    bf16 = mybir.dt.bfloat16
    fp8 = mybir.dt.float8e4
    ts = bass.ts
    DR = mybir.MatmulPerfMode.DoubleRow
    Exp = mybir.ActivationFunctionType.Exp

    nc = bacc.Bacc("TRN2", target_bir_lowering=False, debug=False)

    mT_d = nc.dram_tensor("mT", [BPC, 128, N], bf16, kind="ExternalInput")
    mn_d = nc.dram_tensor("mn", [BPC, N, H], bf16, kind="ExternalInput")
    # [b, c, p, 6 slots (s*3+t), n] fp8, partition-major for 6KB/row DMA
    bias_d = nc.dram_tensor("biasq", [BPC, NCH, 128, 6, N], fp8,
                            kind="ExternalInput")
    wq_d = nc.dram_tensor("wq", [H, H], f32, kind="ExternalInput")
    wk_d = nc.dram_tensor("wk", [H, H], f32, kind="ExternalInput")
    wv_d = nc.dram_tensor("wv", [H, H], f32, kind="ExternalInput")
    bq_d = nc.dram_tensor("bq", [H, 1], f32, kind="ExternalInput")
    bk_d = nc.dram_tensor("bk", [H, 1], f32, kind="ExternalInput")
    bv_d = nc.dram_tensor("bv", [1, H], bf16, kind="ExternalInput")
    ii_d = nc.dram_tensor("ii", [128, 2, 128], fp8, kind="ExternalInput")
    out_d = nc.dram_tensor("out", [BPC, N, H], bf16, kind="ExternalOutput")

    mn_r = mn_d.rearrange("b (i p) h -> b p i h", p=128)
    out_r = out_d.rearrange("b (i p) h -> b p i h", p=128)

    with tile.TileContext(nc) as tc:
        with (
            tc.tile_pool(name="const", bufs=1) as const,
            tc.tile_pool(name="cb", bufs=1) as cbp,
            tc.tile_pool(name="sb", bufs=2) as sb,
            tc.tile_pool(name="epool", bufs=2) as epool,
            tc.tile_pool(name="work", bufs=6) as work,
            tc.tile_pool(name="pqk", bufs=2, space="PSUM") as pqk,
            tc.tile_pool(name="po", bufs=3, space="PSUM") as pop,
        ):
            # ---- constants ----
            wq_f = const.tile([128, 128], f32)
            wk_f = const.tile([128, 128], f32)
            wv_f = const.tile([128, 128], f32)
            nc.scalar.dma_start(out=wq_f, in_=wq_d[:, :])
            nc.scalar.dma_start(out=wk_f, in_=wk_d[:, :])
            nc.scalar.dma_start(out=wv_f, in_=wv_d[:, :])
            wq_b = const.tile([128, 128], bf16)
            wk_b = const.tile([128, 128], bf16)
            wv_b = const.tile([128, 128], bf16)
            nc.vector.tensor_copy(wq_b, wq_f)
            nc.vector.tensor_copy(wk_b, wk_f)
            nc.vector.tensor_copy(wv_b, wv_f)
            bq_s = const.tile([128, 1], f32)
            bk_s = const.tile([128, 1], f32)
            nc.scalar.dma_start(out=bq_s, in_=bq_d[:, :])
            nc.scalar.dma_start(out=bk_s, in_=bk_d[:, :])
            ii = const.tile([128, 2, 128], fp8)
            nc.scalar.dma_start(out=ii, in_=ii_d[:, :, :])
            # bv broadcast across partitions: [1,128] dram -> [128,128]
            bvb = const.tile([128, 128], bf16)
            bv_ap = bv_d[:, :]
            bv_bcast = bass.AP(
                tensor=bv_ap.tensor,
                offset=bv_ap.offset,
                ap=[[0, 128]] + list(bv_ap.ap[1:]),
            )
            nc.gpsimd.dma_start(out=bvb, in_=bv_bcast)
            negC = const.tile([128, 1], f32)
            nc.vector.memset(negC, -CSHIFT)

            # ---- per-batch persistent tiles ----
            cbufs, kids, vaugs, mbs = [], [], [], []
            for b in range(BPC):
                cb0 = cbp.tile([128, 7, N], fp8, name=f"cb{b}_0")
                cb1 = cbp.tile([128, 7, N], fp8, name=f"cb{b}_1")
                cbufs.append((cb0, cb1))
                kid = cbp.tile([128, NB, 2, 128], fp8, name=f"kid{b}")
                # identity into plane 1 of every j slot (DMA, off the
                # critical engines)
                for j in range(NB):
                    nc.gpsimd.dma_start(out=kid[:, j, 1, :], in_=ii_d[:, 0, :])
                kids.append(kid)
                va = cbp.tile([128, NB, 132], fp8, name=f"va{b}")
                nc.vector.memset(va[:, :, 128:129], 1.0)
                vaugs.append(va)

                mT_b = sb.tile([128, N], bf16, name=f"mT{b}", tag="mT")
                nc.scalar.dma_start(out=mT_b, in_=mT_d[b])
                mn_t = sb.tile([128, NB, H], bf16, name=f"mn{b}", tag="mn")
                nc.scalar.dma_start(out=mn_t, in_=mn_r[b])
                mb_t = sb.tile([128, NB, H], bf16, name=f"mb{b}", tag="mb")
                for i in range(NB):
                    nc.gpsimd.tensor_add(mb_t[:, i], mn_t[:, i], bvb)
                mbs.append((mT_b, mb_t))

            # first two bias chunks of batch 0 start streaming immediately
            # (before their cbuf slot-0 qT writes, to avoid any WAW stall
            # later: DMA first, qT write second, scores wait on both)
            nc.sync.dma_start(out=cbufs[0][0][:, 1:7, :], in_=bias_d[0, 0])
            nc.sync.dma_start(out=cbufs[0][1][:, 1:7, :], in_=bias_d[0, 1])

            for b in range(BPC):
                mT_b, mb_t = mbs[b]
                cb0, cb1 = cbufs[b]
                kid = kids[b]
                va = vaugs[b]

                # ---- projections ----
                ps_q = pqk.tile([128, N], f32, name=f"ps_q{b}", tag="pqk")
                nc.tensor.matmul(ps_q[:, 0:512], lhsT=wq_b, rhs=mT_b[:, 0:512],
                                 start=True, stop=True)
                nc.tensor.matmul(ps_q[:, 512:1024], lhsT=wq_b,
                                 rhs=mT_b[:, 512:1024], start=True, stop=True)
                # qT (fp8) into slot 0 of both chunk buffers
                nc.vector.tensor_scalar_add(cb0[:, 0, :], ps_q, bq_s)
                nc.vector.tensor_scalar_add(cb1[:, 0, :], ps_q, bq_s)

                ps_k = pqk.tile([128, N], f32, name=f"ps_k{b}", tag="pqk")
                nc.tensor.matmul(ps_k[:, 0:512], lhsT=wk_b, rhs=mT_b[:, 0:512],
                                 start=True, stop=True)
                nc.tensor.matmul(ps_k[:, 512:1024], lhsT=wk_b,
                                 rhs=mT_b[:, 512:1024], start=True, stop=True)
                # kT (fp8) into plane 0 of kid, strided over j slots
                kin = bass.AP(tensor=ps_k.tensor, offset=ps_k.offset,
                              ap=[list(ps_k.ap[0]), [128, NB], [1, 128]])
                kout_base = kid[:, 0, 0, :]
                kout = bass.AP(tensor=kout_base.tensor, offset=kout_base.offset,
                               ap=[list(kout_base.ap[0]), [256, NB], [1, 128]])
                nc.vector.tensor_scalar_add(kout, kin, bk_s)

                # ---- v (natural [m, h] layout, fp8, no bv) ----
                ps_v = pqk.tile([128, N], f32, name=f"ps_v{b}", tag="pqk")
                for i in range(NB):
                    nc.tensor.matmul(ps_v[:, ts(i, 128)],
                                     lhsT=mT_b[:, ts(i, 128)], rhs=wv_b,
                                     start=True, stop=True,
                                     skip_group_check=True)
                for i in range(NB):
                    nc.scalar.copy(va[:, i, 0:128], ps_v[:, ts(i, 128)])

                # ---- PV accumulators: 8 n-blocks in 3 PSUM banks ----
                ps_os = [
                    pop.tile([128, 3, 132], f32, name=f"ps_o{b}_{p}", tag="po")
                    for p in range(3)
                ]

                # ---- chunks: scores + bias via fp8 DoubleRow, exp, PV ----
                for c in range(NCH):
                    cb = (cb0, cb1)[c % 2]
                    if not (b == 0 and c < 2):
                        nc.sync.dma_start(out=cb[:, 1:7, :], in_=bias_d[b, c])
                    E = epool.tile([128, 2, N], fp8, name=f"E{b}_{c}", tag="E")
                    for s in range(2):
                        j = 2 * c + s
                        ps_s = pqk.tile([128, N], f32, name=f"ps_s{b}_{j}",
                                        tag="pqk")
                        base = cb[:, 0, 0]
                        for h in range(2):
                            # DR1: (kT_j | I) x (qT | b0_j)
                            rhs1 = bass.AP(
                                tensor=base.tensor,
                                offset=base.offset + 512 * h,
                                ap=[list(base.ap[0]), [(1 + 3 * s) * N, 2],
                                    [1, 512]])
                            nc.tensor.matmul(ps_s[:, ts(h, 512)],
                                             lhsT=kid[:, j], rhs=rhs1,
                                             start=True, stop=False,
                                             perf_mode=DR)
                            # DR2: (I | I) x (b1_j | b2_j)
                            rhs2 = bass.AP(
                                tensor=base.tensor,
                                offset=base.offset + (2 + 3 * s) * N + 512 * h,
                                ap=[list(base.ap[0]), [N, 2], [1, 512]])
                            nc.tensor.matmul(ps_s[:, ts(h, 512)],
                                             lhsT=ii, rhs=rhs2,
                                             start=False, stop=True,
                                             perf_mode=DR)
                        nc.scalar.activation(out=E[:, s], in_=ps_s, func=Exp,
                                             bias=negC, scale=SCALE)
                    for i in range(NB):
                        nc.tensor.matmul(
                            ps_os[i // 3][:, i % 3, 0:129],
                            lhsT=E[:, :, ts(i, 128)],
                            rhs=va[:, 2 * c:2 * c + 2, 0:129],
                            start=(c == 0 and i % 3 == 0),
                            stop=(c == NCH - 1),
                            perf_mode=DR, skip_group_check=True)

                # ---- normalization + residual + store ----
                ob = sb.tile([128, NB, H], bf16, name=f"ob{b}", tag="ob")
                for i in range(NB):
                    ps_o = ps_os[i // 3][:, i % 3]
                    r = work.tile([128, 1], f32, name=f"r{b}_{i}", tag="r")
                    nc.vector.reciprocal(r, ps_o[:, 128:129])
                    o1 = work.tile([128, 128], f32, name=f"o1_{b}_{i}",
                                   tag="o1")
                    if i % 2 == 0:
                        nc.scalar.mul(o1, ps_o[:, 0:128], r)
                    else:
                        nc.vector.tensor_scalar_mul(o1, ps_o[:, 0:128], r)
                    nc.gpsimd.tensor_add(ob[:, i], o1, mb_t[:, i])
                    if i % 2 == 1:
                        nc.scalar.dma_start(out=out_r[b, :, i - 1:i + 1],
                                            in_=ob[:, i - 1:i + 1])

    nc.compile()
    _CACHE["nc"] = nc
    return nc


def _shard_inputs(m, adj, dist, coulomb, Wq, bq, Wk, bk, Wv, bv):
    import ml_dtypes

    e4 = ml_dtypes.float8_e4m3
    bf = ml_dtypes.bfloat16

    wqT = np.ascontiguousarray(Wq.T).astype(np.float32)
    wkT = np.ascontiguousarray(Wk.T).astype(np.float32)
    wvT = np.ascontiguousarray(Wv.T).astype(np.float32)
    bq_s = bq.astype(np.float32).reshape(H, 1)
    bk_s = bk.astype(np.float32).reshape(H, 1)
    bv_s = bv.astype(bf).reshape(1, H)

    mT = np.ascontiguousarray(np.swapaxes(m, 1, 2)).astype(bf)
    mn_b = np.ascontiguousarray(m).astype(bf)

    # bias: transpose to [n_key-major], scale by sqrt(H), fp8, interleave to
    # [B, NCH, 128, 6(s*3+t), N] partition-major
    stack = np.stack([np.swapaxes(t, 1, 2) for t in (adj, dist, coulomb)],
                     axis=2)                       # [B, Nk, 3, Nq]
    stack = stack.reshape(B, NCH, 2, 128, 3, N)    # [B, c, s, p, t, n]
    stack = stack.transpose(0, 1, 3, 2, 4, 5)      # [B, c, p, s, t, n]
    biasq = np.ascontiguousarray(stack * INV).astype(e4)
    biasq = biasq.reshape(B, NCH, 128, 6, N)

    I8 = np.eye(128, dtype=np.float32).astype(e4)
    ii = np.ascontiguousarray(np.stack([I8, I8], axis=1))

    in_maps = []
    for c in range(NCORES):
        sl = slice(c * BPC, (c + 1) * BPC)
        in_maps.append({
            "mT": mT[sl],
            "mn": mn_b[sl],
            "biasq": biasq[sl],
            "wq": wqT, "wk": wkT, "wv": wvT,
            "bq": bq_s, "bk": bk_s, "bv": bv_s,
            "ii": ii,
        })
    return in_maps


def run(trace=False, **inputs):
    _install_ntff_hook()
    from concourse.bass_utils import run_bass_kernel_spmd

    nc = _build()
    in_maps = _shard_inputs(**inputs)
    try:
        res = run_bass_kernel_spmd(nc, in_maps, core_ids=list(range(NCORES)),
                                   trace=trace)
    except Exception:
        # transient device errors (NRT_EXEC_UNIT_UNRECOVERABLE) have been
        # observed on this fabric; one retry usually succeeds
        res = run_bass_kernel_spmd(nc, in_maps, core_ids=list(range(NCORES)),
                                   trace=trace)
    out = np.concatenate([r["out"] for r in res.results], axis=0)
    return out.astype(np.float32), res


def kernel(**inputs):
    inputs = {k: np.asarray(v) for k, v in inputs.items()}
    out, _ = run(trace=False, **inputs)
    return out


# revision 6
# speedup vs baseline: 1.2869x; 1.1943x over previous
"""AtomAttention Trainium2 kernel (fp8 DoubleRow, v3).

reference:
    bias = adj + dist + coulomb                      # [B, N, N]
    q = m @ Wq.T + bq; k = m @ Wk.T + bk; v = m @ Wv.T + bv
    attn = softmax(q @ k.T / sqrt(H) + bias, axis=-1)
    out  = attn @ v + m                              # [B, N, H]

B=16, N=1024, H=128.  Data-parallel over batch: 2 batches per core on 8
NeuronCores.  Bias tensors dominate HBM traffic -> shipped pre-scaled by
sqrt(H) in fp8e4m3 (6 MB/core vs 24 MB f32); the 1/sqrt(H) is re-applied
inside the exp (exp(scale*psum - C)); the global shift C keeps E in fp8
range and cancels in softmax.

Engine plan:
  - fp8 DoubleRow matmuls (2 contractions per pass):
      DR1: (kT_j | I) x (qT | b0_j)   -> k.T q + b0
      DR2: (I | I)    x (b1_j | b2_j) -> + b1 + b2
    PV DoubleRows pair consecutive j-blocks; denominator comes from a
    ones column in v.
  - exp split: even j on ACT (activation Exp -> fp8), odd j on DVE via
    the Schraudolph bit trick (bits = clamp(round(A*psum + B)) written
    as uint8 = fp8e4m3 encoding of exp).  fp8 rounding noise dwarfs the
    ~3% trick error; softmax normalization absorbs it (sim: 5.35e-3 vs
    5.32e-3 all-ACT, gate 2e-2).
  - descriptor generation (~0.8us per dma_start, serial per engine
    sequencer) is spread across all five rings; the m/w loads no longer
    queue behind each other, and the 24-slot mega bias tile per batch
    (slot 0 = qT) kills all WAR stalls between chunks.
  - kT lands contiguously in kid[:,0,:]; kid[:,1,:] = 8 identity copies
    DMA'd from a host tensor, so DR lhsT slices are natural tile views.
"""

import sys
import types

import numpy as np

B, N, H = 16, 1024, 128
NB = N // 128   # 8 row blocks
NCH = 4         # chunks per batch, 2 blocks each
BPC = 2         # batches per core
NCORES = 8
SCALE = float(1.0 / np.sqrt(np.float32(H)))   # 0.08838835
INV = float(np.sqrt(np.float32(H)))           # 11.313708
CSHIFT = 3.25                                 # global exp shift
LOG2E = 1.4426950408889634
EXP_A = SCALE * LOG2E * 8.0                   # 1.0201033
EXP_B = 8.0 * (7.0 - CSHIFT * LOG2E) - 0.4    # Schraudolph, RNE convert

_CACHE = {}


def _install_ntff_hook():
    """The agent image's antenv lacks axon_hooks; register the NTFF
    profiling hook manually so trace=True yields exec_time_ns."""
    if "antenv.axon_hooks" in sys.modules:
        return
    try:
        import trn_agent_boot.trn_boot as tb

        hook = tb._ntff_profile_via_ctypes("/opt/axon/libaxon_pjrt.so")
    except Exception:
        hook = None
    mod = types.ModuleType("antenv.axon_hooks")
    mod.get_axon_ntff_profile_hook = lambda: hook
    mod.set_axon_ntff_profile_hook = lambda h: None
    sys.modules["antenv.axon_hooks"] = mod


def _build():
    if "nc" in _CACHE:
        return _CACHE["nc"]
    import concourse.bass as bass
    from concourse import bacc, mybir, tile

    f32 = mybir.dt.float32
    bf16 = mybir.dt.bfloat16
    fp8 = mybir.dt.float8e4
    u8 = mybir.dt.uint8
    ts = bass.ts
    DR = mybir.MatmulPerfMode.DoubleRow
    Exp = mybir.ActivationFunctionType.Exp
    Alu = mybir.AluOpType

    nc = bacc.Bacc("TRN2", target_bir_lowering=False, debug=False)

    mT_d = nc.dram_tensor("mT", [BPC, 128, N], bf16, kind="ExternalInput")
    mn_d = nc.dram_tensor("mn", [BPC, N, H], bf16, kind="ExternalInput")
    # [b, c, p, 6 slots (s*3+t), n] fp8, partition-major for 6KB/row DMA
    bias_d = nc.dram_tensor("biasq", [BPC, NCH, 128, 6, N], fp8,
                            kind="ExternalInput")
    ii8_d = nc.dram_tensor("ii8", [128, N], fp8, kind="ExternalInput")
    wq_d = nc.dram_tensor("wq", [H, H], f32, kind="ExternalInput")
    wk_d = nc.dram_tensor("wk", [H, H], f32, kind="ExternalInput")
    wv_d = nc.dram_tensor("wv", [H, H], f32, kind="ExternalInput")
    bq_d = nc.dram_tensor("bq", [H, 1], f32, kind="ExternalInput")
    bk_d = nc.dram_tensor("bk", [H, 1], f32, kind="ExternalInput")
    bv_d = nc.dram_tensor("bv", [1, H], bf16, kind="ExternalInput")
    out_d = nc.dram_tensor("out", [BPC, N, H], bf16, kind="ExternalOutput")

    mn_r = mn_d.rearrange("b (i p) h -> b p i h", p=128)
    out_r = out_d.rearrange("b (i p) h -> b p i h", p=128)

    with tile.TileContext(nc) as tc:
        with (
            tc.tile_pool(name="const", bufs=1) as const,
            tc.tile_pool(name="cb", bufs=1) as cbp,
            tc.tile_pool(name="sb", bufs=2) as sb,
            tc.tile_pool(name="epool", bufs=2) as epool,
            tc.tile_pool(name="work", bufs=6) as work,
            tc.tile_pool(name="pqk", bufs=2, space="PSUM") as pqk,
            tc.tile_pool(name="po", bufs=3, space="PSUM") as pop,
        ):
            # ---- allocate big per-batch tiles ----
            btiles = [cbp.tile([128, 25, N], fp8, name=f"bt{b}")
                      for b in range(BPC)]
            kids = [cbp.tile([128, 2, N], fp8, name=f"kid{b}")
                    for b in range(BPC)]
            vaugs = [cbp.tile([128, NB, 132], fp8, name=f"va{b}")
                     for b in range(BPC)]
            mT_ts = [sb.tile([128, N], bf16, name=f"mT{b}", tag="mT")
                     for b in range(BPC)]
            mn_ts = [sb.tile([128, NB, H], bf16, name=f"mn{b}", tag="mn")
                     for b in range(BPC)]
            mb_ts = [sb.tile([128, NB, H], bf16, name=f"mb{b}", tag="mb")
                     for b in range(BPC)]

            # ---- descriptor generation spread across the 3 DMA rings ----
            # scalar ring: mT first (PE's first dependency), then weights
            nc.scalar.dma_start(out=mT_ts[0], in_=mT_d[0])
            nc.scalar.dma_start(out=mT_ts[1], in_=mT_d[1])
            wq_f = const.tile([128, 128], f32)
            wk_f = const.tile([128, 128], f32)
            wv_f = const.tile([128, 128], f32)
            bq_s = const.tile([128, 1], f32)
            bk_s = const.tile([128, 1], f32)
            nc.scalar.dma_start(out=wk_f, in_=wk_d[:, :])
            nc.scalar.dma_start(out=wq_f, in_=wq_d[:, :])
            nc.scalar.dma_start(out=wv_f, in_=wv_d[:, :])
            nc.scalar.dma_start(out=bk_s, in_=bk_d[:, :])
            nc.scalar.dma_start(out=bq_s, in_=bq_d[:, :])
            # sync ring: the 8 bias chunk streams
            for b in range(BPC):
                for c in range(NCH):
                    nc.sync.dma_start(out=btiles[b][:, 1 + 6 * c:7 + 6 * c, :],
                                      in_=bias_d[b, c])
            # gpsimd ring: mn, identity planes, bv broadcast
            bvb = const.tile([128, 128], bf16)
            bv_ap = bv_d[:, :]
            bv_bcast = bass.AP(
                tensor=bv_ap.tensor,
                offset=bv_ap.offset,
                ap=[[0, 128]] + list(bv_ap.ap[1:]),
            )
            nc.gpsimd.dma_start(out=mn_ts[0], in_=mn_r[0])
            nc.gpsimd.dma_start(out=kids[0][:, 1, :], in_=ii8_d[:, :])
            nc.gpsimd.dma_start(out=bvb, in_=bv_bcast)
            nc.gpsimd.dma_start(out=mn_ts[1], in_=mn_r[1])
            nc.gpsimd.dma_start(out=kids[1][:, 1, :], in_=ii8_d[:, :])

            # ---- DVE-side constants ----
            wk_b = const.tile([128, 128], bf16)
            wq_b = const.tile([128, 128], bf16)
            wv_b = const.tile([128, 128], bf16)
            nc.vector.tensor_copy(wk_b, wk_f)
            nc.vector.tensor_copy(wq_b, wq_f)
            nc.vector.tensor_copy(wv_b, wv_f)
            negC = const.tile([128, 1], f32)
            nc.vector.memset(negC, -CSHIFT)
            for b in range(BPC):
                nc.vector.memset(vaugs[b][:, :, 128:129], 1.0)

            # residual prep on gpsimd (idle early)
            for b in range(BPC):
                for i in range(NB):
                    nc.gpsimd.tensor_add(mb_ts[b][:, i], mn_ts[b][:, i], bvb)

            # ---- projections, both batches up front ----
            for b in range(BPC):
                mT_b = mT_ts[b]
                ps_k = pqk.tile([128, N], f32, name=f"ps_k{b}", tag="pqk")
                nc.tensor.matmul(ps_k[:, 0:512], lhsT=wk_b, rhs=mT_b[:, 0:512],
                                 start=True, stop=True)
                nc.tensor.matmul(ps_k[:, 512:1024], lhsT=wk_b,
                                 rhs=mT_b[:, 512:1024], start=True, stop=True)
                # kT (fp8) contiguous into kid plane 0
                nc.vector.tensor_scalar_add(kids[b][:, 0, :], ps_k, bk_s)

                ps_q = pqk.tile([128, N], f32, name=f"ps_q{b}", tag="pqk")
                nc.tensor.matmul(ps_q[:, 0:512], lhsT=wq_b, rhs=mT_b[:, 0:512],
                                 start=True, stop=True)
                nc.tensor.matmul(ps_q[:, 512:1024], lhsT=wq_b,
                                 rhs=mT_b[:, 512:1024], start=True, stop=True)
                # qT (fp8) into slot 0 of the mega bias tile
                nc.vector.tensor_scalar_add(btiles[b][:, 0, :], ps_q, bq_s)

                ps_v = pqk.tile([128, N], f32, name=f"ps_v{b}", tag="pqk")
                for i in range(NB):
                    nc.tensor.matmul(ps_v[:, ts(i, 128)],
                                     lhsT=mT_b[:, ts(i, 128)], rhs=wv_b,
                                     start=(i % 4 == 0), stop=(i % 4 == 3),
                                     skip_group_check=True)
                for i in range(NB):
                    nc.vector.tensor_copy(vaugs[b][:, i, 0:128],
                                          ps_v[:, ts(i, 128)])

            # ---- chunk loop: scores + bias (DR), exp, PV (DR) ----
            for b in range(BPC):
                bt = btiles[b]
                kid = kids[b]
                va = vaugs[b]
                ps_os = [
                    pop.tile([128, 3, 132], f32, name=f"ps_o{b}_{p}", tag="po")
                    for p in range(3)
                ]
                base = bt[:, 0, 0]
                pstride = list(base.ap[0])
                # (I | I) weights pair: two identity blocks of kid plane 1
                ibase = kid[:, 1, 0]
                ii_w = bass.AP(tensor=ibase.tensor, offset=ibase.offset,
                               ap=[list(ibase.ap[0]), [128, 2], [1, 128]])
                for c in range(NCH):
                    E = epool.tile([128, 2, N], fp8, name=f"E{b}_{c}", tag="E")
                    for s in range(2):
                        j = 2 * c + s
                        slot = 1 + 6 * c + 3 * s
                        ps_s = pqk.tile([128, N], f32, name=f"ps_s{b}_{j}",
                                        tag="pqk")
                        for h in range(2):
                            rhs1 = bass.AP(
                                tensor=base.tensor,
                                offset=base.offset + 512 * h,
                                ap=[pstride, [slot * N, 2], [1, 512]])
                            nc.tensor.matmul(ps_s[:, ts(h, 512)],
                                             lhsT=kid[:, :, ts(j, 128)],
                                             rhs=rhs1, start=True, stop=False,
                                             perf_mode=DR)
                            rhs2 = bass.AP(
                                tensor=base.tensor,
                                offset=base.offset + (slot + 1) * N + 512 * h,
                                ap=[pstride, [N, 2], [1, 512]])
                            nc.tensor.matmul(ps_s[:, ts(h, 512)],
                                             lhsT=ii_w, rhs=rhs2,
                                             start=False, stop=True,
                                             perf_mode=DR)
                        if s == 0:
                            nc.scalar.activation(out=E[:, 0], in_=ps_s,
                                                 func=Exp, bias=negC,
                                                 scale=SCALE)
                        else:
                            # Schraudolph: uint8 bits == fp8e4m3(exp(...))
                            nc.vector.tensor_scalar(
                                out=E[:, 1].bitcast(u8), in0=ps_s,
                                scalar1=EXP_A, scalar2=EXP_B,
                                op0=Alu.mult, op1=Alu.add)
                    for i in range(NB):
                        nc.tensor.matmul(
                            ps_os[i // 3][:, i % 3, 0:129],
                            lhsT=E[:, :, ts(i, 128)],
                            rhs=va[:, 2 * c:2 * c + 2, 0:129],
                            start=(c == 0 and i % 3 == 0),
                            stop=(c == NCH - 1),
                            perf_mode=DR, skip_group_check=True)

                # ---- normalization + residual + one store per batch ----
                ob = sb.tile([128, NB, H], bf16, name=f"ob{b}", tag="ob")
                mb_t = mb_ts[b]
                for i in range(NB):
                    ps_o = ps_os[i // 3][:, i % 3]
                    r = work.tile([128, 1], f32, name=f"r{b}_{i}", tag="r")
                    nc.vector.reciprocal(r, ps_o[:, 128:129])
                    o1 = work.tile([128, 128], f32, name=f"o1_{b}_{i}",
                                   tag="o1")
                    nc.scalar.mul(o1, ps_o[:, 0:128], r)
                    nc.vector.tensor_add(ob[:, i], o1, mb_t[:, i])
                nc.gpsimd.dma_start(out=out_r[b], in_=ob)

    nc.compile()
    _CACHE["nc"] = nc
    return nc


def _shard_inputs(m, adj, dist, coulomb, Wq, bq, Wk, bk, Wv, bv):
    import ml_dtypes

    e4 = ml_dtypes.float8_e4m3
    bf = ml_dtypes.bfloat16

    wqT = np.ascontiguousarray(Wq.T).astype(np.float32)
    wkT = np.ascontiguousarray(Wk.T).astype(np.float32)
    wvT = np.ascontiguousarray(Wv.T).astype(np.float32)
    bq_s = bq.astype(np.float32).reshape(H, 1)
    bk_s = bk.astype(np.float32).reshape(H, 1)
    bv_s = bv.astype(bf).reshape(1, H)

    mT = np.ascontiguousarray(np.swapaxes(m, 1, 2)).astype(bf)
    mn_b = np.ascontiguousarray(m).astype(bf)

    # bias: transpose to key-major, scale by sqrt(H), fp8, interleave to
    # [B, NCH, 128, 6(s*3+t), N] partition-major
    stack = np.stack([np.swapaxes(t, 1, 2) for t in (adj, dist, coulomb)],
                     axis=2)                       # [B, Nk, 3, Nq]
    stack = stack.reshape(B, NCH, 2, 128, 3, N)    # [B, c, s, p, t, n]
    stack = stack.transpose(0, 1, 3, 2, 4, 5)      # [B, c, p, s, t, n]
    biasq = np.ascontiguousarray(stack * INV).astype(e4)
    biasq = biasq.reshape(B, NCH, 128, 6, N)

    I8 = np.eye(128, dtype=np.float32).astype(e4)
    ii8 = np.ascontiguousarray(np.tile(I8, (1, NB)))

    in_maps = []
    for c in range(NCORES):
        sl = slice(c * BPC, (c + 1) * BPC)
        in_maps.append({
            "mT": mT[sl],
            "mn": mn_b[sl],
            "biasq": biasq[sl],
            "wq": wqT, "wk": wkT, "wv": wvT,
            "bq": bq_s, "bk": bk_s, "bv": bv_s,
            "ii8": ii8,
        })
    return in_maps


def run(trace=False, **inputs):
    _install_ntff_hook()
    from concourse.bass_utils import run_bass_kernel_spmd

    nc = _build()
    in_maps = _shard_inputs(**inputs)
    try:
        res = run_bass_kernel_spmd(nc, in_maps, core_ids=list(range(NCORES)),
                                   trace=trace)
    except Exception:
        # transient device errors (NRT_EXEC_UNIT_UNRECOVERABLE) have been
        # observed on this fabric; one retry usually succeeds
        res = run_bass_kernel_spmd(nc, in_maps, core_ids=list(range(NCORES)),
                                   trace=trace)
    out = np.concatenate([r["out"] for r in res.results], axis=0)
    return out.astype(np.float32), res


def kernel(**inputs):
    inputs = {k: np.asarray(v) for k, v in inputs.items()}
    out, _ = run(trace=False, **inputs)
    return out


# revision 7
# speedup vs baseline: 1.4416x; 1.1202x over previous
"""AtomAttention Trainium2 kernel (fp8 DoubleRow, v4).

reference:
    bias = adj + dist + coulomb                      # [B, N, N]
    q = m @ Wq.T + bq; k = m @ Wk.T + bk; v = m @ Wv.T + bv
    attn = softmax(q @ k.T / sqrt(H) + bias, axis=-1)
    out  = attn @ v + m                              # [B, N, H]

B=16, N=1024, H=128.  Data-parallel over batch: 2 batches per core on 8
NeuronCores.  Bias tensors dominate HBM traffic -> shipped pre-scaled by
sqrt(H) in fp8e4m3 (6 MB/core vs 24 MB f32); the 1/sqrt(H) is re-applied
inside the exp; a global shift C keeps E in fp8 range and cancels in
softmax.

Engine plan:
  - fp8 DoubleRow matmuls (two contractions per pass):
      DR1: (kT_j | I) x (qT | b0_j)   -> k.T q + b0
      DR2: (I | I)    x (b1_j | b2_j) -> + b1 + b2
    PV DoubleRows pair consecutive j-blocks; softmax denominator comes
    from a ones column in v.
  - scores live in single-bank [128,512] PSUM tiles (bufs=5) so several
    j-halves are in flight; exp of each half alternates between ACT
    (activation Exp) and DVE (Schraudolph bit trick: clamp(round(A*psum
    + B)) written as uint8 == fp8e4m3 bits of exp).  Total rel err
    ~5.4e-3 (gate 2e-2).
  - PV for chunk c issues after scores of chunk c+1 so the in-order PE
    never waits on exp; PV accumulates across chunks in 3 PSUM banks.
  - mega 25-slot bias tile per batch (slot 0 = qT) -> chunk DMAs never
    have WAR hazards; descriptor generation is spread across the three
    DMA rings (sync=bias, scalar=weights+mT, gpsimd=mn/identity/out).
"""

import sys
import types

import numpy as np

B, N, H = 16, 1024, 128
NB = N // 128   # 8 row blocks
NCH = 4         # chunks per batch, 2 blocks each
BPC = 2         # batches per core
NCORES = 8
SCALE = float(1.0 / np.sqrt(np.float32(H)))   # 0.08838835
INV = float(np.sqrt(np.float32(H)))           # 11.313708
CSHIFT = 3.25                                 # global exp shift
LOG2E = 1.4426950408889634
EXP_A = SCALE * LOG2E * 8.0                   # 1.0201033
EXP_B = 8.0 * (7.0 - CSHIFT * LOG2E) - 0.4    # Schraudolph, RNE convert

_CACHE = {}


def _install_ntff_hook():
    """The agent image's antenv lacks axon_hooks; register the NTFF
    profiling hook manually so trace=True yields exec_time_ns."""
    if "antenv.axon_hooks" in sys.modules:
        return
    try:
        import trn_agent_boot.trn_boot as tb

        hook = tb._ntff_profile_via_ctypes("/opt/axon/libaxon_pjrt.so")
    except Exception:
        hook = None
    mod = types.ModuleType("antenv.axon_hooks")
    mod.get_axon_ntff_profile_hook = lambda: hook
    mod.set_axon_ntff_profile_hook = lambda h: None
    sys.modules["antenv.axon_hooks"] = mod


def _build():
    if "nc" in _CACHE:
        return _CACHE["nc"]
    import concourse.bass as bass
    from concourse import bacc, mybir, tile

    f32 = mybir.dt.float32
    bf16 = mybir.dt.bfloat16
    fp8 = mybir.dt.float8e4
    u8 = mybir.dt.uint8
    ts = bass.ts
    DR = mybir.MatmulPerfMode.DoubleRow
    Exp = mybir.ActivationFunctionType.Exp
    Alu = mybir.AluOpType

    nc = bacc.Bacc("TRN2", target_bir_lowering=False, debug=False)

    mT_d = nc.dram_tensor("mT", [BPC, 128, N], bf16, kind="ExternalInput")
    mn_d = nc.dram_tensor("mn", [BPC, N, H], bf16, kind="ExternalInput")
    # [b, c, p, 6 slots (s*3+t), n] fp8, partition-major for 6KB/row DMA
    bias_d = nc.dram_tensor("biasq", [BPC, NCH, 128, 6, N], fp8,
                            kind="ExternalInput")
    ii8_d = nc.dram_tensor("ii8", [128, N], fp8, kind="ExternalInput")
    wq_d = nc.dram_tensor("wq", [H, H], f32, kind="ExternalInput")
    wk_d = nc.dram_tensor("wk", [H, H], f32, kind="ExternalInput")
    wv_d = nc.dram_tensor("wv", [H, H], f32, kind="ExternalInput")
    bq_d = nc.dram_tensor("bq", [H, 1], f32, kind="ExternalInput")
    bk_d = nc.dram_tensor("bk", [H, 1], f32, kind="ExternalInput")
    bv_d = nc.dram_tensor("bv", [1, H], bf16, kind="ExternalInput")
    out_d = nc.dram_tensor("out", [BPC, N, H], bf16, kind="ExternalOutput")

    mn_r = mn_d.rearrange("b (i p) h -> b p i h", p=128)
    out_r = out_d.rearrange("b (i p) h -> b p i h", p=128)

    with tile.TileContext(nc) as tc:
        with (
            tc.tile_pool(name="const", bufs=1) as const,
            tc.tile_pool(name="cb", bufs=1) as cbp,
            tc.tile_pool(name="sb", bufs=2) as sb,
            tc.tile_pool(name="epool", bufs=2) as epool,
            tc.tile_pool(name="work", bufs=6) as work,
            tc.tile_pool(name="pqk", bufs=5, space="PSUM") as pqk,
            tc.tile_pool(name="po", bufs=3, space="PSUM") as pop,
        ):
            # ---- allocate big per-batch tiles ----
            btiles = [cbp.tile([128, 25, N], fp8, name=f"bt{b}")
                      for b in range(BPC)]
            kids = [cbp.tile([128, 2, N], fp8, name=f"kid{b}")
                    for b in range(BPC)]
            vaugs = [cbp.tile([128, NB, 132], fp8, name=f"va{b}")
                     for b in range(BPC)]
            mT_ts = [sb.tile([128, N], bf16, name=f"mT{b}", tag="mT")
                     for b in range(BPC)]
            mn_ts = [sb.tile([128, NB, H], bf16, name=f"mn{b}", tag="mn")
                     for b in range(BPC)]
            mb_ts = [sb.tile([128, NB, H], bf16, name=f"mb{b}", tag="mb")
                     for b in range(BPC)]

            # ---- descriptor generation spread across the 3 DMA rings ----
            # scalar ring: wk first (unblocks the weight casts), mT early
            wq_f = const.tile([128, 128], f32)
            wk_f = const.tile([128, 128], f32)
            wv_f = const.tile([128, 128], f32)
            bq_s = const.tile([128, 1], f32)
            bk_s = const.tile([128, 1], f32)
            nc.scalar.dma_start(out=wk_f, in_=wk_d[:, :])
            nc.scalar.dma_start(out=mT_ts[0], in_=mT_d[0])
            nc.scalar.dma_start(out=wq_f, in_=wq_d[:, :])
            nc.scalar.dma_start(out=wv_f, in_=wv_d[:, :])
            nc.scalar.dma_start(out=mT_ts[1], in_=mT_d[1])
            nc.scalar.dma_start(out=bk_s, in_=bk_d[:, :])
            nc.scalar.dma_start(out=bq_s, in_=bq_d[:, :])
            # sync ring: the 8 bias chunk streams
            for b in range(BPC):
                for c in range(NCH):
                    nc.sync.dma_start(out=btiles[b][:, 1 + 6 * c:7 + 6 * c, :],
                                      in_=bias_d[b, c])
            # gpsimd ring: identity planes, mn, bv broadcast
            bvb = const.tile([128, 128], bf16)
            bv_ap = bv_d[:, :]
            bv_bcast = bass.AP(
                tensor=bv_ap.tensor,
                offset=bv_ap.offset,
                ap=[[0, 128]] + list(bv_ap.ap[1:]),
            )
            nc.gpsimd.dma_start(out=kids[0][:, 1, :], in_=ii8_d[:, :])
            nc.gpsimd.dma_start(out=kids[1][:, 1, :], in_=ii8_d[:, :])
            nc.gpsimd.dma_start(out=mn_ts[0], in_=mn_r[0])
            nc.gpsimd.dma_start(out=bvb, in_=bv_bcast)
            nc.gpsimd.dma_start(out=mn_ts[1], in_=mn_r[1])

            # ---- DVE-side constants ----
            wk_b = const.tile([128, 128], bf16)
            wq_b = const.tile([128, 128], bf16)
            wv_b = const.tile([128, 128], bf16)
            nc.vector.tensor_copy(wk_b, wk_f)
            nc.vector.tensor_copy(wq_b, wq_f)
            nc.vector.tensor_copy(wv_b, wv_f)
            negC = const.tile([128, 1], f32)
            nc.vector.memset(negC, -CSHIFT)
            for b in range(BPC):
                nc.vector.memset(vaugs[b][:, :, 128:129], 1.0)

            # residual prep on gpsimd (idle early)
            for b in range(BPC):
                for i in range(NB):
                    nc.gpsimd.tensor_add(mb_ts[b][:, i], mn_ts[b][:, i], bvb)

            # ---- projections, both batches up front ----
            for b in range(BPC):
                mT_b = mT_ts[b]
                for h in range(2):
                    ps_kh = pqk.tile([128, 512], f32, name=f"ps_k{b}_{h}",
                                     tag="pqk")
                    nc.tensor.matmul(ps_kh, lhsT=wk_b, rhs=mT_b[:, ts(h, 512)],
                                     start=True, stop=True)
                    nc.vector.tensor_scalar_add(
                        kids[b][:, 0, ts(h, 512)], ps_kh, bk_s)
                for h in range(2):
                    ps_qh = pqk.tile([128, 512], f32, name=f"ps_q{b}_{h}",
                                     tag="pqk")
                    nc.tensor.matmul(ps_qh, lhsT=wq_b, rhs=mT_b[:, ts(h, 512)],
                                     start=True, stop=True)
                    nc.vector.tensor_scalar_add(
                        btiles[b][:, 0, ts(h, 512)], ps_qh, bq_s)
                for h in range(2):
                    ps_vh = pqk.tile([128, 512], f32, name=f"ps_v{b}_{h}",
                                     tag="pqk")
                    for i in range(4):
                        nc.tensor.matmul(ps_vh[:, ts(i, 128)],
                                         lhsT=mT_b[:, ts(4 * h + i, 128)],
                                         rhs=wv_b,
                                         start=(i == 0), stop=(i == 3),
                                         skip_group_check=True)
                    for i in range(4):
                        nc.vector.tensor_copy(vaugs[b][:, 4 * h + i, 0:128],
                                              ps_vh[:, ts(i, 128)])

            # ---- chunk loop: scores (DR) -> exp (ACT/DVE) -> PV one
            # chunk behind, so the in-order PE never blocks on exp ----
            for b in range(BPC):
                bt = btiles[b]
                kid = kids[b]
                va = vaugs[b]
                ps_os = [
                    pop.tile([128, 3, 132], f32, name=f"ps_o{b}_{p}", tag="po")
                    for p in range(3)
                ]
                base = bt[:, 0, 0]
                pstride = list(base.ap[0])
                # (I | I) weights pair: two identity blocks of kid plane 1
                ibase = kid[:, 1, 0]
                ii_w = bass.AP(tensor=ibase.tensor, offset=ibase.offset,
                               ap=[list(ibase.ap[0]), [128, 2], [1, 128]])
                Es = []

                def scores(c):
                    E = epool.tile([128, 2, N], fp8, name=f"E{b}_{c}", tag="E")
                    for s in range(2):
                        j = 2 * c + s
                        slot = 1 + 6 * c + 3 * s
                        for h in range(2):
                            ps_s = pqk.tile([128, 512], f32,
                                            name=f"ps_s{b}_{j}_{h}",
                                            tag="pqk")
                            rhs1 = bass.AP(
                                tensor=base.tensor,
                                offset=base.offset + 512 * h,
                                ap=[pstride, [slot * N, 2], [1, 512]])
                            nc.tensor.matmul(ps_s, lhsT=kid[:, :, ts(j, 128)],
                                             rhs=rhs1, start=True, stop=False,
                                             perf_mode=DR)
                            rhs2 = bass.AP(
                                tensor=base.tensor,
                                offset=base.offset + (slot + 1) * N + 512 * h,
                                ap=[pstride, [N, 2], [1, 512]])
                            nc.tensor.matmul(ps_s, lhsT=ii_w, rhs=rhs2,
                                             start=False, stop=True,
                                             perf_mode=DR)
                            eh = E[:, s, ts(h, 512)]
                            if h == 0:
                                nc.scalar.activation(out=eh, in_=ps_s,
                                                     func=Exp, bias=negC,
                                                     scale=SCALE)
                            else:
                                nc.vector.tensor_scalar(
                                    out=eh.bitcast(u8), in0=ps_s,
                                    scalar1=EXP_A, scalar2=EXP_B,
                                    op0=Alu.mult, op1=Alu.add)
                    Es.append(E)

                def pv(c):
                    E = Es[c]
                    for i in range(NB):
                        nc.tensor.matmul(
                            ps_os[i // 3][:, i % 3, 0:129],
                            lhsT=E[:, :, ts(i, 128)],
                            rhs=va[:, 2 * c:2 * c + 2, 0:129],
                            start=(c == 0 and i % 3 == 0),
                            stop=(c == NCH - 1),
                            perf_mode=DR, skip_group_check=True)

                scores(0)
                for c in range(1, NCH):
                    scores(c)
                    pv(c - 1)
                pv(NCH - 1)

                # ---- normalization + residual + one store per batch ----
                ob = sb.tile([128, NB, H], bf16, name=f"ob{b}", tag="ob")
                mb_t = mb_ts[b]
                for i in range(NB):
                    ps_o = ps_os[i // 3][:, i % 3]
                    r = work.tile([128, 1], f32, name=f"r{b}_{i}", tag="r")
                    nc.vector.reciprocal(r, ps_o[:, 128:129])
                    o1 = work.tile([128, 128], f32, name=f"o1_{b}_{i}",
                                   tag="o1")
                    nc.scalar.mul(o1, ps_o[:, 0:128], r)
                    nc.vector.tensor_add(ob[:, i], o1, mb_t[:, i])
                nc.gpsimd.dma_start(out=out_r[b], in_=ob)

    nc.compile()
    _CACHE["nc"] = nc
    return nc


def _shard_inputs(m, adj, dist, coulomb, Wq, bq, Wk, bk, Wv, bv):
    import ml_dtypes

    e4 = ml_dtypes.float8_e4m3
    bf = ml_dtypes.bfloat16

    wqT = np.ascontiguousarray(Wq.T).astype(np.float32)
    wkT = np.ascontiguousarray(Wk.T).astype(np.float32)
    wvT = np.ascontiguousarray(Wv.T).astype(np.float32)
    bq_s = bq.astype(np.float32).reshape(H, 1)
    bk_s = bk.astype(np.float32).reshape(H, 1)
    bv_s = bv.astype(bf).reshape(1, H)

    mT = np.ascontiguousarray(np.swapaxes(m, 1, 2)).astype(bf)
    mn_b = np.ascontiguousarray(m).astype(bf)

    # bias: transpose to key-major, scale by sqrt(H), fp8, interleave to
    # [B, NCH, 128, 6(s*3+t), N] partition-major
    stack = np.stack([np.swapaxes(t, 1, 2) for t in (adj, dist, coulomb)],
                     axis=2)                       # [B, Nk, 3, Nq]
    stack = stack.reshape(B, NCH, 2, 128, 3, N)    # [B, c, s, p, t, n]
    stack = stack.transpose(0, 1, 3, 2, 4, 5)      # [B, c, p, s, t, n]
    biasq = np.ascontiguousarray(stack * INV).astype(e4)
    biasq = biasq.reshape(B, NCH, 128, 6, N)

    I8 = np.eye(128, dtype=np.float32).astype(e4)
    ii8 = np.ascontiguousarray(np.tile(I8, (1, NB)))

    in_maps = []
    for c in range(NCORES):
        sl = slice(c * BPC, (c + 1) * BPC)
        in_maps.append({
            "mT": mT[sl],
            "mn": mn_b[sl],
            "biasq": biasq[sl],
            "wq": wqT, "wk": wkT, "wv": wvT,
            "bq": bq_s, "bk": bk_s, "bv": bv_s,
            "ii8": ii8,
        })
    return in_maps


def run(trace=False, **inputs):
    _install_ntff_hook()
    from concourse.bass_utils import run_bass_kernel_spmd

    nc = _build()
    in_maps = _shard_inputs(**inputs)
    try:
        res = run_bass_kernel_spmd(nc, in_maps, core_ids=list(range(NCORES)),
                                   trace=trace)
    except Exception:
        # transient device errors (NRT_EXEC_UNIT_UNRECOVERABLE) have been
        # observed on this fabric; one retry usually succeeds
        res = run_bass_kernel_spmd(nc, in_maps, core_ids=list(range(NCORES)),
                                   trace=trace)
    out = np.concatenate([r["out"] for r in res.results], axis=0)
    return out.astype(np.float32), res


def kernel(**inputs):
    inputs = {k: np.asarray(v) for k, v in inputs.items()}
    out, _ = run(trace=False, **inputs)
    return out


# revision 8
# speedup vs baseline: 1.7672x; 1.2259x over previous
"""AtomAttention Trainium2 kernel (fp8 DoubleRow, v4).

reference:
    bias = adj + dist + coulomb                      # [B, N, N]
    q = m @ Wq.T + bq; k = m @ Wk.T + bk; v = m @ Wv.T + bv
    attn = softmax(q @ k.T / sqrt(H) + bias, axis=-1)
    out  = attn @ v + m                              # [B, N, H]

B=16, N=1024, H=128.  Data-parallel over batch: 2 batches per core on 8
NeuronCores.  Bias tensors dominate HBM traffic -> shipped pre-scaled by
sqrt(H) in fp8e4m3 (6 MB/core vs 24 MB f32); the 1/sqrt(H) is re-applied
inside the exp; a global shift C keeps E in fp8 range and cancels in
softmax.

Engine plan:
  - fp8 DoubleRow matmuls (two contractions per pass):
      DR1: (kT_j | I) x (qT | b0_j)   -> k.T q + b0
      DR2: (I | I)    x (b1_j | b2_j) -> + b1 + b2
    PV DoubleRows pair consecutive j-blocks; softmax denominator comes
    from a ones column in v.
  - scores live in single-bank [128,512] PSUM tiles (bufs=5) so several
    j-halves are in flight; exp of each half alternates between ACT
    (activation Exp) and DVE (Schraudolph bit trick: clamp(round(A*psum
    + B)) written as uint8 == fp8e4m3 bits of exp).  Total rel err
    ~5.4e-3 (gate 2e-2).
  - PV for chunk c issues after scores of chunk c+1 so the in-order PE
    never waits on exp; PV accumulates across chunks in 3 PSUM banks.
  - mega 25-slot bias tile per batch (slot 0 = qT) -> chunk DMAs never
    have WAR hazards; descriptor generation is spread across the three
    DMA rings (sync=bias, scalar=weights+mT, gpsimd=mn/identity/out).
"""

import sys
import types

import numpy as np

B, N, H = 16, 1024, 128
NB = N // 128   # 8 row blocks
NCH = 4         # chunks per batch, 2 blocks each
BPC = 2         # batches per core
NCORES = 8
SCALE = float(1.0 / np.sqrt(np.float32(H)))   # 0.08838835
INV = float(np.sqrt(np.float32(H)))           # 11.313708
CSHIFT = 3.25                                 # global exp shift
LOG2E = 1.4426950408889634
EXP_A = SCALE * LOG2E * 8.0                   # 1.0201033
EXP_B = 8.0 * (7.0 - CSHIFT * LOG2E) - 0.4    # Schraudolph, RNE convert

_CACHE = {}


def _install_ntff_hook():
    """The agent image's antenv lacks axon_hooks; register the NTFF
    profiling hook manually so trace=True yields exec_time_ns."""
    if "antenv.axon_hooks" in sys.modules:
        return
    try:
        import trn_agent_boot.trn_boot as tb

        hook = tb._ntff_profile_via_ctypes("/opt/axon/libaxon_pjrt.so")
    except Exception:
        hook = None
    mod = types.ModuleType("antenv.axon_hooks")
    mod.get_axon_ntff_profile_hook = lambda: hook
    mod.set_axon_ntff_profile_hook = lambda h: None
    sys.modules["antenv.axon_hooks"] = mod


def _build():
    if "nc" in _CACHE:
        return _CACHE["nc"]
    import concourse.bass as bass
    from concourse import bacc, mybir, tile

    f32 = mybir.dt.float32
    bf16 = mybir.dt.bfloat16
    fp8 = mybir.dt.float8e4
    u8 = mybir.dt.uint8
    ts = bass.ts
    DR = mybir.MatmulPerfMode.DoubleRow
    Exp = mybir.ActivationFunctionType.Exp
    Alu = mybir.AluOpType

    nc = bacc.Bacc("TRN2", target_bir_lowering=False, debug=False)

    mT_d = nc.dram_tensor("mT", [BPC, 128, N], bf16, kind="ExternalInput")
    mn_d = nc.dram_tensor("mn", [BPC, N, H], bf16, kind="ExternalInput")
    # [b, c, p, 6 slots (s*3+t), n] fp8, partition-major for 6KB/row DMA
    bias_d = nc.dram_tensor("biasq", [BPC, NCH, 128, 6, N], fp8,
                            kind="ExternalInput")
    ii8_d = nc.dram_tensor("ii8", [128, N], fp8, kind="ExternalInput")
    wq_d = nc.dram_tensor("wq", [H, H], bf16, kind="ExternalInput")
    wk_d = nc.dram_tensor("wk", [H, H], bf16, kind="ExternalInput")
    wv_d = nc.dram_tensor("wv", [H, H], bf16, kind="ExternalInput")
    bq_d = nc.dram_tensor("bq", [H, 1], f32, kind="ExternalInput")
    bk_d = nc.dram_tensor("bk", [H, 1], f32, kind="ExternalInput")
    bv_d = nc.dram_tensor("bv", [1, H], bf16, kind="ExternalInput")
    out_d = nc.dram_tensor("out", [BPC, N, H], bf16, kind="ExternalOutput")

    mn_r = mn_d.rearrange("b (i p) h -> b p i h", p=128)
    out_r = out_d.rearrange("b (i p) h -> b p i h", p=128)

    with tile.TileContext(nc) as tc:
        with (
            tc.tile_pool(name="const", bufs=1) as const,
            tc.tile_pool(name="cb", bufs=1) as cbp,
            tc.tile_pool(name="sb", bufs=2) as sb,
            tc.tile_pool(name="epool", bufs=2) as epool,
            tc.tile_pool(name="work", bufs=6) as work,
            tc.tile_pool(name="pqk", bufs=5, space="PSUM") as pqk,
            tc.tile_pool(name="po", bufs=3, space="PSUM") as pop,
        ):
            # ---- allocate big per-batch tiles ----
            btiles = [cbp.tile([128, 25, N], fp8, name=f"bt{b}")
                      for b in range(BPC)]
            kids = [cbp.tile([128, 2, N], fp8, name=f"kid{b}")
                    for b in range(BPC)]
            vaugs = [cbp.tile([128, NB, 132], fp8, name=f"va{b}")
                     for b in range(BPC)]
            mT_ts = [sb.tile([128, N], bf16, name=f"mT{b}", tag="mT")
                     for b in range(BPC)]
            mn_ts = [sb.tile([128, NB, H], bf16, name=f"mn{b}", tag="mn")
                     for b in range(BPC)]
            mb_ts = [sb.tile([128, NB, H], bf16, name=f"mb{b}", tag="mb")
                     for b in range(BPC)]

            # ---- descriptor generation spread across the 3 DMA rings ----
            # scalar ring: bf16 weights (tiny, land first)
            wk_b = const.tile([128, 128], bf16)
            wq_b = const.tile([128, 128], bf16)
            wv_b = const.tile([128, 128], bf16)
            bq_s = const.tile([128, 1], f32)
            bk_s = const.tile([128, 1], f32)
            nc.scalar.dma_start(out=wk_b, in_=wk_d[:, :])
            nc.scalar.dma_start(out=wq_b, in_=wq_d[:, :])
            nc.scalar.dma_start(out=wv_b, in_=wv_d[:, :])
            nc.scalar.dma_start(out=bk_s, in_=bk_d[:, :])
            nc.scalar.dma_start(out=bq_s, in_=bq_d[:, :])
            # sync ring: mT ahead of the bias flood (queue FIFO), then the
            # bias chunks in half-chunk pieces for deeper queue pipelining
            nc.sync.dma_start(out=mT_ts[0], in_=mT_d[0])
            nc.sync.dma_start(out=mT_ts[1], in_=mT_d[1])
            for b in range(BPC):
                for c in range(NCH):
                    for s in range(2):
                        sl = 1 + 6 * c + 3 * s
                        nc.sync.dma_start(
                            out=btiles[b][:, sl:sl + 3, :],
                            in_=bias_d[b, c, :, 3 * s:3 * s + 3, :])
            # gpsimd ring: identity planes, mn, bv broadcast
            bvb = const.tile([128, 128], bf16)
            bv_ap = bv_d[:, :]
            bv_bcast = bass.AP(
                tensor=bv_ap.tensor,
                offset=bv_ap.offset,
                ap=[[0, 128]] + list(bv_ap.ap[1:]),
            )
            nc.gpsimd.dma_start(out=kids[0][:, 1, :], in_=ii8_d[:, :])
            nc.gpsimd.dma_start(out=kids[1][:, 1, :], in_=ii8_d[:, :])
            nc.gpsimd.dma_start(out=mn_ts[0], in_=mn_r[0])
            nc.gpsimd.dma_start(out=mn_ts[1], in_=mn_r[1])
            nc.gpsimd.dma_start(out=bvb, in_=bv_bcast)

            # ---- DVE-side constants ----
            negC = const.tile([128, 1], f32)
            nc.vector.memset(negC, -CSHIFT)
            for b in range(BPC):
                nc.vector.memset(vaugs[b][:, :, 128:129], 1.0)

            # residual prep on gpsimd (idle early)
            for b in range(BPC):
                for i in range(NB):
                    nc.gpsimd.tensor_add(mb_ts[b][:, i], mn_ts[b][:, i], bvb)

            # ---- projections, both batches up front ----
            for b in range(BPC):
                mT_b = mT_ts[b]
                for h in range(2):
                    ps_kh = pqk.tile([128, 512], f32, name=f"ps_k{b}_{h}",
                                     tag="pqk")
                    nc.tensor.matmul(ps_kh, lhsT=wk_b, rhs=mT_b[:, ts(h, 512)],
                                     start=True, stop=True)
                    nc.vector.tensor_scalar_add(
                        kids[b][:, 0, ts(h, 512)], ps_kh, bk_s)
                for h in range(2):
                    ps_qh = pqk.tile([128, 512], f32, name=f"ps_q{b}_{h}",
                                     tag="pqk")
                    nc.tensor.matmul(ps_qh, lhsT=wq_b, rhs=mT_b[:, ts(h, 512)],
                                     start=True, stop=True)
                    nc.vector.tensor_scalar_add(
                        btiles[b][:, 0, ts(h, 512)], ps_qh, bq_s)
                for h in range(2):
                    ps_vh = pqk.tile([128, 512], f32, name=f"ps_v{b}_{h}",
                                     tag="pqk")
                    for i in range(4):
                        nc.tensor.matmul(ps_vh[:, ts(i, 128)],
                                         lhsT=mT_b[:, ts(4 * h + i, 128)],
                                         rhs=wv_b,
                                         start=(i == 0), stop=(i == 3),
                                         skip_group_check=True)
                    for i in range(4):
                        nc.vector.tensor_copy(vaugs[b][:, 4 * h + i, 0:128],
                                              ps_vh[:, ts(i, 128)])

            # ---- chunk loop: scores (DR) -> exp (ACT/DVE) -> PV one
            # chunk behind, so the in-order PE never blocks on exp ----
            for b in range(BPC):
                bt = btiles[b]
                kid = kids[b]
                va = vaugs[b]
                ps_os = [
                    pop.tile([128, 3, 132], f32, name=f"ps_o{b}_{p}", tag="po")
                    for p in range(3)
                ]
                base = bt[:, 0, 0]
                pstride = list(base.ap[0])
                # (I | I) weights pair: two identity blocks of kid plane 1
                ibase = kid[:, 1, 0]
                ii_w = bass.AP(tensor=ibase.tensor, offset=ibase.offset,
                               ap=[list(ibase.ap[0]), [128, 2], [1, 128]])
                Es = []

                def scores(c):
                    E = epool.tile([128, 2, N], fp8, name=f"E{b}_{c}", tag="E")
                    for s in range(2):
                        j = 2 * c + s
                        slot = 1 + 6 * c + 3 * s
                        pss = [pqk.tile([128, 512], f32,
                                        name=f"ps_s{b}_{j}_{h}", tag="pqk")
                               for h in range(2)]
                        for h in range(2):
                            rhs1 = bass.AP(
                                tensor=base.tensor,
                                offset=base.offset + 512 * h,
                                ap=[pstride, [slot * N, 2], [1, 512]])
                            nc.tensor.matmul(pss[h],
                                             lhsT=kid[:, :, ts(j, 128)],
                                             rhs=rhs1, start=True, stop=False,
                                             perf_mode=DR,
                                             skip_group_check=True)
                        for h in range(2):
                            rhs2 = bass.AP(
                                tensor=base.tensor,
                                offset=base.offset + (slot + 1) * N + 512 * h,
                                ap=[pstride, [N, 2], [1, 512]])
                            nc.tensor.matmul(pss[h], lhsT=ii_w, rhs=rhs2,
                                             start=False, stop=True,
                                             perf_mode=DR,
                                             skip_group_check=True)
                        for h in range(2):
                            eh = E[:, s, ts(h, 512)]
                            if h == 0:
                                nc.scalar.activation(out=eh, in_=pss[h],
                                                     func=Exp, bias=negC,
                                                     scale=SCALE)
                            else:
                                nc.vector.tensor_scalar(
                                    out=eh.bitcast(u8), in0=pss[h],
                                    scalar1=EXP_A, scalar2=EXP_B,
                                    op0=Alu.mult, op1=Alu.add)
                    Es.append(E)

                def pv(c):
                    E = Es[c]
                    for i in range(NB):
                        nc.tensor.matmul(
                            ps_os[i // 3][:, i % 3, 0:129],
                            lhsT=E[:, :, ts(i, 128)],
                            rhs=va[:, 2 * c:2 * c + 2, 0:129],
                            start=(c == 0 and i % 3 == 0),
                            stop=(c == NCH - 1),
                            perf_mode=DR, skip_group_check=True)

                scores(0)
                for c in range(1, NCH):
                    scores(c)
                    pv(c - 1)
                pv(NCH - 1)

                # ---- normalization + residual + one store per batch ----
                ob = sb.tile([128, NB, H], bf16, name=f"ob{b}", tag="ob")
                mb_t = mb_ts[b]
                for i in range(NB):
                    ps_o = ps_os[i // 3][:, i % 3]
                    r = work.tile([128, 1], f32, name=f"r{b}_{i}", tag="r")
                    nc.vector.reciprocal(r, ps_o[:, 128:129])
                    o1 = work.tile([128, 128], f32, name=f"o1_{b}_{i}",
                                   tag="o1")
                    nc.scalar.mul(o1, ps_o[:, 0:128], r)
                    nc.vector.tensor_add(ob[:, i], o1, mb_t[:, i])
                    if i % 2 == 1:
                        nc.gpsimd.dma_start(out=out_r[b, :, i - 1:i + 1],
                                            in_=ob[:, i - 1:i + 1])

    nc.compile()
    _CACHE["nc"] = nc
    return nc


def _shard_inputs(m, adj, dist, coulomb, Wq, bq, Wk, bk, Wv, bv):
    import ml_dtypes

    e4 = ml_dtypes.float8_e4m3
    bf = ml_dtypes.bfloat16

    wqT = np.ascontiguousarray(Wq.T).astype(bf)
    wkT = np.ascontiguousarray(Wk.T).astype(bf)
    wvT = np.ascontiguousarray(Wv.T).astype(bf)
    bq_s = bq.astype(np.float32).reshape(H, 1)
    bk_s = bk.astype(np.float32).reshape(H, 1)
    bv_s = bv.astype(bf).reshape(1, H)

    mT = np.ascontiguousarray(np.swapaxes(m, 1, 2)).astype(bf)
    mn_b = np.ascontiguousarray(m).astype(bf)

    # bias: transpose to key-major, scale by sqrt(H), fp8, interleave to
    # [B, NCH, 128, 6(s*3+t), N] partition-major
    stack = np.stack([np.swapaxes(t, 1, 2) for t in (adj, dist, coulomb)],
                     axis=2)                       # [B, Nk, 3, Nq]
    stack = stack.reshape(B, NCH, 2, 128, 3, N)    # [B, c, s, p, t, n]
    stack = stack.transpose(0, 1, 3, 2, 4, 5)      # [B, c, p, s, t, n]
    biasq = np.ascontiguousarray(stack * INV).astype(e4)
    biasq = biasq.reshape(B, NCH, 128, 6, N)

    I8 = np.eye(128, dtype=np.float32).astype(e4)
    ii8 = np.ascontiguousarray(np.tile(I8, (1, NB)))

    in_maps = []
    for c in range(NCORES):
        sl = slice(c * BPC, (c + 1) * BPC)
        in_maps.append({
            "mT": mT[sl],
            "mn": mn_b[sl],
            "biasq": biasq[sl],
            "wq": wqT, "wk": wkT, "wv": wvT,
            "bq": bq_s, "bk": bk_s, "bv": bv_s,
            "ii8": ii8,
        })
    return in_maps


def run(trace=False, **inputs):
    _install_ntff_hook()
    from concourse.bass_utils import run_bass_kernel_spmd

    nc = _build()
    in_maps = _shard_inputs(**inputs)
    try:
        res = run_bass_kernel_spmd(nc, in_maps, core_ids=list(range(NCORES)),
                                   trace=trace)
    except Exception:
        # transient device errors (NRT_EXEC_UNIT_UNRECOVERABLE) have been
        # observed on this fabric; one retry usually succeeds
        res = run_bass_kernel_spmd(nc, in_maps, core_ids=list(range(NCORES)),
                                   trace=trace)
    out = np.concatenate([r["out"] for r in res.results], axis=0)
    return out.astype(np.float32), res


def kernel(**inputs):
    inputs = {k: np.asarray(v) for k, v in inputs.items()}
    out, _ = run(trace=False, **inputs)
    return out
